# revision 37
# baseline (speedup 1.0000x reference)
"""ExpertNet (moe_routing) Trainium2 Bass kernel, v2.

Data-parallel over 8 NeuronCores: batch N=32768 split into 8 shards of 4096.
All parameters replicated.  Per-core pipeline per 512-sample block:

  X^T --(PE)--> h^T --(PE, z+dist fused)--> [z^T | -2mu.z] --(DVE)--> q
     --(DMA bcast)--> per-pair q rows --(DVE)--> z*q
     --(PE, row-packed K=64 pairs)--> expert hidden --(ACT/DVE relu)-->
     --(PE, M=32 accumulate)--> q-weighted logits sum = preds^T
     --(ACT copy + DVE normalize)--> OUTT[32, NS] --> DRAM (host transposes)

Differences vs v1 (232.6 us): PE matmuls per block cut 114 -> 103 and the
DMA-issue path unclogged:
  * -2mu^T z is folded into the z-layer matmuls (stationary widened to 80
    cols: 64 z outputs + 16 dist partials over the same h moving data).
  * dist constant (1+|mu|^2-2mu.bz) enters via DVE tensor_scalar_add; the
    reciprocal uses the 1-op reciprocal_approx_fast (~18 significant bits)
    instead of the ~6x slower iterative InstReciprocal.
  * the per-pair q broadcasts (8 PE selector matmuls in v1) are ONE DMA with
    a 0-stride partition dim on the source AP, issued from the idle gpsimd
    queue; the normalizer broadcast (o132 matmul in v1) likewise.
  * X loads are 2 batched DMAs per block (v1: 8) and weight loads are 4
    DMAs on the scalar/vector queues, so SP.SEQ descriptor generation
    (~2.4us per dma_start) no longer serializes the pipeline.
  * preds are stored transposed ([32, NS] per core); the host un-transposes.
    This drops the DVE 32x32 transpose and makes the store 32 contiguous
    2KB descriptors.

Matmuls run in float32r (full-rate fp32, ~11-bit mantissa).  q = 1/(1+dist)
is folded into z BEFORE the expert MLP (relu(q*x) = q*relu(x) for q>0), so
the soft combine is plain PSUM accumulation; q-normalization (1/sum q) is a
final per-column scale of preds^T.

b1 != 0 falls back to the v1 kernel (setup_inputs uses b1=0).
"""

import numpy as np

N, D, H_ENC, NZ, KE, H_EXP, C = 32768, 1024, 512, 64, 16, 256, 10
NCORES = 8
NS = N // NCORES          # samples per core
NPAIR = KE // 2           # expert pairs (row-packed)

# WR (f32r) column layout
WENC_C = 0                # 8 dc chunks x 512
WZM2_C = 4096             # 4 hc chunks x 128 (16 dist, 48 pad, 64 z)
W1_C = WZM2_C + 4 * 128   # 8 pairs x 256
W2_C = W1_C + NPAIR * H_EXP   # 32 chunks x 32
O64_C = W2_C + KE * 2 * 32    # ones [64, 16]
O16_C = O64_C + KE            # ones [16, 1]
B2_C = O16_C + 1              # b2 padded [16, 32]
RCOLS = B2_C + 32
# WF (f32) column layout: benc 0..3, bz 4, bd1 5
FCOLS = 6

_CACHE = {}
LAST_RESULTS = None


def _build(has_b1: bool, cfg: dict | None = None):
    if has_b1:
        return _build_legacy(True, cfg)
    defaults = dict(penc=2, pexp=4, pmisc=1, ppred=1, hbufs=5, ehbufs=9,
                    zqbufs=3, xbufs=2, zbufs=3, qbufs=3, qbbufs=4, obufs=2,
                    qsbufs=2, nact=5, repeat=1, W=512, ahead=2, xsplit=8)
    cfg = {**defaults, **(cfg or {})}
    import concourse.bacc as bacc
    import concourse.mybir as mybir
    from concourse import tile

    F32 = mybir.dt.float32
    F32R = mybir.dt.float32r
    BF16 = mybir.dt.bfloat16
    AF = mybir.ActivationFunctionType

    W = cfg["W"]
    NBLK = NS // W
    nc = bacc.Bacc("TRN2", target_bir_lowering=False, debug=False,
                   num_devices=NCORES)

    # XP[ib, p, dc*W + c] = X[ib*W + c, dc*128 + p]: each partition's block
    # slice is one contiguous 16KB run -> 128 DMA descriptors per block.
    XP = nc.dram_tensor("XP", [NS // cfg["W"], 128, 8 * cfg["W"]], F32R,
                        kind="ExternalInput")
    WR = nc.dram_tensor("WR", [128, RCOLS], F32R, kind="ExternalInput")
    WF = nc.dram_tensor("WF", [128, FCOLS], F32, kind="ExternalInput")
    OUTT = nc.dram_tensor("OUTT", [32, NS], F32, kind="ExternalOutput")
    if cfg.get("dbg"):
        DBG = {n: nc.dram_tensor(n, [128, cfg["W"]], F32, kind="ExternalOutput")
               for n in ("Dzt2", "Dqr", "Dqb", "Dprb", "Dzq", "Dti")}
        DBG["Dqs"] = nc.dram_tensor("Dqs", [1, KE * cfg["W"]], F32,
                                    kind="ExternalOutput")

    with tile.TileContext(nc) as tc, nc.allow_low_precision(
        reason="float32r tiles feed the PE; rounding is ~1e-4 relative"
    ):
        with (
            tc.tile_pool(name="wpool", bufs=1) as wp,
            tc.tile_pool(name="xpool", bufs=cfg["xbufs"]) as xp,
            tc.tile_pool(name="hpool", bufs=cfg["hbufs"]) as hp,
            tc.tile_pool(name="zpool", bufs=cfg["zbufs"]) as zp,
            tc.tile_pool(name="qpool", bufs=cfg["qbufs"]) as qp,
            tc.tile_pool(name="qbpool", bufs=cfg["qbbufs"]) as qbp,
            tc.tile_pool(name="qspool", bufs=cfg["qsbufs"]) as qsp,
            tc.tile_pool(name="zqpool", bufs=cfg["zqbufs"]) as zqp,
            tc.tile_pool(name="ehpool", bufs=cfg["ehbufs"]) as ehp,
            tc.tile_pool(name="opool", bufs=cfg["obufs"]) as top,
            tc.tile_pool(name="pbig", bufs=cfg["pbig"], space="PSUM") as pbig,
            tc.tile_pool(name="pmisc", bufs=cfg["pmisc"], space="PSUM") as pmisc,
            tc.tile_pool(name="ppred", bufs=cfg["ppred"], space="PSUM") as ppred,
        ):
            # ---- weights: 4 DMAs, off the SP queue so block-0 X loads
            # (on SP) issue concurrently -------------------------------
            wr = wp.tile([128, RCOLS], F32R, name="WR_sb")
            wf = wp.tile([128, FCOLS], F32, name="WF_sb")
            # wenc dc-chunk 0 first so the first encoder matmul can start
            # ~3us in; X block-0 issues concurrently on the SP queue.
            for s in range(4):
                nc.scalar.dma_start(wr[:, s * 1024:(s + 1) * 1024],
                                    WR[:, s * 1024:(s + 1) * 1024])
            nc.gpsimd.dma_start(out=wf[:], in_=WF[:])
            nc.gpsimd.dma_start(out=wr[:, 4096:RCOLS], in_=WR[:, 4096:RCOLS])

            def front(ib):
                n0 = ib * W
                xt = xp.tile([128, 8 * W], F32R, tag="xt")
                cs = 8 * W // cfg["xsplit"]
                for s in range(cfg["xsplit"]):
                    nc.sync.dma_start(
                        xt[:, s * cs:(s + 1) * cs],
                        XP[ib, :, s * cs:(s + 1) * cs],
                    )

                # encoder: hT[hc] = relu(Wenc^T X^T + benc)
                hts = []
                for hc in range(4):
                    ph = pbig.tile([128, W], F32, tag="pbig")
                    for dc in range(8):
                        nc.tensor.matmul(
                            ph[:],
                            wr[:, dc * 512 + hc * 128: dc * 512 + (hc + 1) * 128],
                            xt[:, dc * W:(dc + 1) * W],
                            start=(dc == 0), stop=(dc == 7),
                        )
                    ht = hp.tile([128, W], F32R, tag="ht")
                    nc.scalar.activation(ht[:], ph[:], AF.Relu,
                                         bias=wf[:, hc:hc + 1])
                    hts.append(ht)

                # z layer fused with -2mu.z: rows 0:64 = z, rows 64:80 = dist
                # partial, all over the same h moving data.
                pze = pmisc.tile([128, W], F32, tag="pmisc")
                for hc in range(4):
                    nc.tensor.matmul(
                        pze[:],
                        wr[:, WZM2_C + hc * 128: WZM2_C + (hc + 1) * 128],
                        hts[hc][:],
                        start=(hc == 0), stop=(hc == 3),
                    )
                zt2 = zp.tile([128, W], F32R, tag="zt2")
                nc.scalar.activation(zt2[0:64], pze[64:128], AF.Identity,
                                     bias=wf[0:64, 4:5])
                nc.scalar.activation(zt2[64:128], pze[64:128], AF.Identity,
                                     bias=wf[0:64, 4:5])
                zsq = zp.tile([64, W], F32R, tag="zsq")
                nc.vector.tensor_mul(zsq[:], zt2[0:64], zt2[0:64])
                # |z|^2 accumulates onto the dist rows of the open psum
                nc.tensor.matmul(pze[0:16], wr[0:64, O64_C:O64_C + KE],
                                 zsq[:], start=False, stop=True,
                                 skip_group_check=True)

                # q = 1/(dist + 1 + |mu|^2): bias via DVE, 1-op approx recip.
                # The custom-DVE recip writes plain f32; a tensor_copy then
                # produces the f32r-rounded view the PE consumes (the BIR
                # verifier requires engine-written f32r matmul operands to be
                # pre-rounded).
                qt = qp.tile([16, W], F32, tag="qt")
                nc.vector.tensor_scalar_add(qt[:], pze[0:16], wf[0:16, 5:6])
                qf = qp.tile([16, W], F32, tag="qf")
                nc.vector.reciprocal_approx_fast(qf[:], qt[:])
                qr = qp.tile([16, W], F32R, tag="qr")
                nc.vector.tensor_copy(qr[:], qf[:])

                # normalizer 1/sum_k q, broadcast to 32 rows for the final
                # per-column scale of preds^T
                pqs = pmisc.tile([1, W], F32, tag="pmisc")
                nc.tensor.matmul(pqs[:], wr[0:16, O16_C:O16_C + 1], qr[:],
                                 start=True, stop=True)
                rqs = qp.tile([1, W], F32, tag="rqs")
                nc.vector.reciprocal_approx_fast(rqs[:], pqs[:])
                prb = qp.tile([32, W], F32, tag="prb")
                nc.gpsimd.partition_broadcast(prb[:], rqs[0:1, :])

                # Gather the 16 q rows into one partition (expert-major along
                # free) so each per-expert broadcast reads from partition 0,
                # which is the only source partition the broadcast ISA allows.
                qs2 = qsp.tile([1, KE * W], BF16, tag="qs2")
                nc.gpsimd.dma_start(
                    out=qs2[:].rearrange("a (k c) -> a k c", c=W),
                    in_=qr[:],
                )
                if cfg.get("dbg") and ib == 0:
                    nc.sync.dma_start(DBG["Dzt2"][:].bitcast(F32R), zt2[:])
                    nc.sync.dma_start(DBG["Dqr"][0:16, :].bitcast(F32R), qr[:])
                    nc.gpsimd.dma_start(out=DBG["Dqs"][:], in_=qs2[:])
                    nc.sync.dma_start(DBG["Dprb"][0:32, :], prb[:])
                return dict(zt2=zt2, qr=qr, qs2=qs2, prb=prb, n0=n0, ib=ib)

            def back(st):
                zt2, qr, qb, prb, n0 = (st["zt2"], st["qr"], st["qb"],
                                        st["prb"], st["n0"])
                pp = ppred.tile([32, W], F32, tag="ppred")
                # b2 term; ALSO zero-fills rows 10-31 exactly (start=True).
                nc.tensor.matmul(pp[:], wr[0:16, B2_C:B2_C + 32], qr[:],
                                 start=True, stop=False)
                ci = 0
                for j in range(NPAIR):
                    zq = zqp.tile([128, W], F32R, tag="zq")
                    nc.vector.tensor_mul(zq[:], zt2[:], qb[:, j * W:(j + 1) * W])
                    if cfg.get("dbg") and st["ib"] == 0 and j == 0:
                        nc.sync.dma_start(DBG["Dzq"][:].bitcast(F32R), zq[:])
                    for hc in range(2):
                        for half in range(2):
                            idx = (2 * j + half) * 2 + hc
                            pe_ = pbig.tile([128, W], F32, tag="pbig")
                            nc.tensor.matmul(
                                pe_[:],
                                wr[64 * half:64 * (half + 1),
                                   W1_C + j * H_EXP + hc * 128:
                                   W1_C + j * H_EXP + (hc + 1) * 128],
                                zq[64 * half:64 * (half + 1), :],
                                start=True, stop=True,
                                tile_position=(64 * half, 0),
                            )
                            eh = ehp.tile([128, W], F32R, tag="eh")
                            if idx % 8 < cfg["nact"]:
                                nc.scalar.activation(eh[:], pe_[:], AF.Relu,
                                                     bias=0.0)
                            else:
                                nc.vector.tensor_scalar_max(eh[:], pe_[:], 0.0)
                            ci += 1
                            nc.tensor.matmul(
                                pp[:],
                                wr[:, W2_C + idx * 32:W2_C + (idx + 1) * 32],
                                eh[:],
                                start=False, stop=(ci == NPAIR * 4),
                                skip_group_check=True,
                            )

                # normalize preds^T by 1/sum q and store (transposed layout)
                ti = top.tile([32, W], F32, tag="ti")
                nc.scalar.activation(ti[:], pp[:], AF.Copy)
                if cfg.get("dbg") and st["ib"] == 0:
                    nc.sync.dma_start(DBG["Dti"][0:32, :], ti[:])
                nc.vector.tensor_mul(ti[:], ti[:], prb[:])
                nc.sync.dma_start(OUTT[:, n0:n0 + W], ti[:])

            A = cfg["ahead"]
            for _rep in range(cfg["repeat"]):
                sts = [front(0)]
                for ib in range(1, min(A, NBLK)):
                    sts.append(front(ib))
                for ib in range(NBLK):
                    if ib + A < NBLK:
                        sts.append(front(ib + A))
                    back(sts[ib])
                sts.clear()

    nc.compile()
    return nc


def _prep(inputs):
    f = lambda a: np.ascontiguousarray(np.asarray(a, dtype=np.float32))
    X, enc_W, enc_b = f(inputs["X"]), f(inputs["enc_W"]), f(inputs["enc_b"])
    z_W, z_b, mu = f(inputs["z_W"]), f(inputs["z_b"]), f(inputs["mu"])
    W1, b1, W2, b2 = f(inputs["W1"]), f(inputs["b1"]), f(inputs["W2"]), f(inputs["b2"])

    has_b1 = bool(np.any(b1))
    if has_b1:
        return _prep_legacy(inputs)

    wr = np.zeros((128, RCOLS), np.float32)
    # encoder: wr[p, dc*512 + h] = enc_W[dc*128 + p, h]
    wr[:, 0:4096] = enc_W.reshape(8, 128, H_ENC).transpose(1, 0, 2).reshape(128, 4096)
    # z + dist: WM2 = z_W @ (-2 mu^T)
    n2mu = -2.0 * mu.T                                   # [NZ, KE]
    wm2 = z_W @ n2mu                                     # [H_ENC, KE]
    for hc in range(4):
        wr[:, WZM2_C + hc * 128: WZM2_C + hc * 128 + 16] = wm2[hc * 128:(hc + 1) * 128]
        wr[:, WZM2_C + hc * 128 + 64: WZM2_C + (hc + 1) * 128] = z_W[hc * 128:(hc + 1) * 128]
    for k in range(KE):
        wr[0:64, W1_C + k * 128: W1_C + (k + 1) * 128] = W1[k][:, 0:128]
        wr[64:128, W1_C + k * 128: W1_C + (k + 1) * 128] = W1[k][:, 128:256]
    for k in range(KE):
        for hc in range(2):
            idx = k * 2 + hc
            wr[:, W2_C + idx * 32: W2_C + idx * 32 + C] = W2[k][hc * 128:(hc + 1) * 128]
    wr[0:64, O64_C:O64_C + KE] = 1.0
    wr[0:16, O16_C] = 1.0
    wr[0:16, B2_C:B2_C + C] = b2

    wfm = np.zeros((128, FCOLS), np.float32)
    wfm[:, 0:4] = enc_b.reshape(4, 128).T
    wfm[0:64, 4] = z_b
    # dist constant: 1 + |mu|^2 - 2 mu.bz  (the -2mu.z fold sees z w/o bias)
    wfm[0:16, 5] = (1.0 + (mu.astype(np.float64) ** 2).sum(axis=1)
                    + (n2mu.astype(np.float64) * z_b[:, None]).sum(axis=0)
                    ).astype(np.float32)

    # XP[ib, p, dc*W + c] = X[n_base + ib*W + c, dc*128 + p]
    W = 512
    in_maps = []
    for c in range(NCORES):
        shard = X[c * NS:(c + 1) * NS]                   # [NS, D]
        xp = np.ascontiguousarray(
            shard.reshape(NS // W, W, 8, 128).transpose(0, 3, 2, 1)
            .reshape(NS // W, 128, 8 * W))
        in_maps.append({"WR": wr, "WF": wfm, "XP": xp})
    return in_maps, False


def kernel(**inputs) -> np.ndarray:
    global LAST_RESULTS
    from concourse.bass_utils import run_bass_kernel_spmd

    in_maps, has_b1 = _prep(inputs)
    if has_b1 not in _CACHE:
        _CACHE[has_b1] = _build(has_b1)
    nc = _CACHE[has_b1]

    res = run_bass_kernel_spmd(nc, in_maps, list(range(NCORES)))
    LAST_RESULTS = res
    if has_b1:
        out = np.concatenate([res.results[c]["OUT"] for c in range(NCORES)],
                             axis=0)
    else:
        out = np.concatenate(
            [res.results[c]["OUTT"][0:C, :].T for c in range(NCORES)], axis=0)
    return np.ascontiguousarray(out, dtype=np.float32)


# ---------------------------------------------------------------------------
# legacy v1 kernel (used only when b1 != 0; setup_inputs has b1 = 0)
# ---------------------------------------------------------------------------

NB = 512


def _build_legacy(has_b1: bool, cfg: dict | None = None):
    defaults = dict(pbig=4, pmisc=1, pqb=2, ppred=1, hbufs=9, ehbufs=10,
                    zqbufs=3, xbufs=3, qb_gpsimd=False, repeat=1, W=NB,
                    ahead=2)
    cfg = {**defaults, **(cfg or {})}
    import concourse.bacc as bacc
    import concourse.mybir as mybir
    from concourse import tile

    F32 = mybir.dt.float32
    F32R = mybir.dt.float32r
    AF = mybir.ActivationFunctionType

    W = cfg["W"]
    NBLK = NS // NB
    nc = bacc.Bacc("TRN2", target_bir_lowering=False, debug=False,
                   num_devices=NCORES)

    XT = nc.dram_tensor("XT", [8, 128, NS], F32R, kind="ExternalInput")
    Wenc = nc.dram_tensor("Wenc", [128, 8 * H_ENC], F32R, kind="ExternalInput")
    Wz = nc.dram_tensor("Wz", [128, 4 * NZ], F32R, kind="ExternalInput")
    W1p = nc.dram_tensor("W1p", [128, NPAIR * H_EXP], F32R, kind="ExternalInput")
    W2c = nc.dram_tensor("W2c", [128, KE * 2 * 32], F32R, kind="ExternalInput")
    NEG2MUT = nc.dram_tensor("NEG2MUT", [NZ, KE], F32R, kind="ExternalInput")
    ONES64 = nc.dram_tensor("ONES64", [NZ, KE], F32R, kind="ExternalInput")
    ONES16 = nc.dram_tensor("ONES16", [KE, 1], F32R, kind="ExternalInput")
    ONES1_32 = nc.dram_tensor("ONES1_32", [1, 32], F32R, kind="ExternalInput")
    ONESN = nc.dram_tensor("ONESN", [1, NB], F32R, kind="ExternalInput")
    E2 = nc.dram_tensor("E2", [KE, NPAIR * 128], F32R, kind="ExternalInput")
    B2PAD = nc.dram_tensor("B2PAD", [KE, 32], F32R, kind="ExternalInput")
    BENC = nc.dram_tensor("BENC", [128, 4], F32, kind="ExternalInput")
    BZ = nc.dram_tensor("BZ", [NZ, 1], F32, kind="ExternalInput")
    BD1 = nc.dram_tensor("BD1", [1, KE], F32R, kind="ExternalInput")
    if has_b1:
        B1C = nc.dram_tensor("B1C", [128, KE * 2], F32, kind="ExternalInput")
        E2S = nc.dram_tensor("E2S", [KE, KE * 128], F32R, kind="ExternalInput")
    OUT = nc.dram_tensor("OUT", [NS, C], F32, kind="ExternalOutput")

    with tile.TileContext(nc) as tc, nc.allow_low_precision(
        reason="float32r tiles feed the PE; rounding is ~1e-4 relative"
    ):
        with (
            tc.tile_pool(name="wpool", bufs=1) as wp,
            tc.tile_pool(name="xpool", bufs=cfg["xbufs"]) as xp,
            tc.tile_pool(name="hpool", bufs=cfg["hbufs"]) as hp,
            tc.tile_pool(name="zpool", bufs=2) as zp,
            tc.tile_pool(name="qpool", bufs=2) as qp,
            tc.tile_pool(name="zqpool", bufs=cfg["zqbufs"]) as zqp,
            tc.tile_pool(name="ehpool", bufs=cfg["ehbufs"]) as ehp,
            tc.tile_pool(name="trpool", bufs=2) as trp,
            tc.tile_pool(name="pbig", bufs=cfg["pbig"], space="PSUM") as pbig,
            tc.tile_pool(name="pmisc", bufs=cfg["pmisc"], space="PSUM") as pmisc,
            tc.tile_pool(name="pqb", bufs=max(cfg["pqb"], 1), space="PSUM") as pqb,
            tc.tile_pool(name="ppred", bufs=cfg["ppred"], space="PSUM") as ppred,
        ):
            def wload(dram, shape, dt):
                t = wp.tile(shape, dt, name=dram.name + "_sb")
                nc.sync.dma_start(t[:], dram[:])
                return t

            wenc = wp.tile([128, 8 * H_ENC], F32R, name="Wenc_sb")
            for dc in range(8):
                nc.sync.dma_start(wenc[:, dc * H_ENC:(dc + 1) * H_ENC],
                                  Wenc[:, dc * H_ENC:(dc + 1) * H_ENC])
            benc = wload(BENC, [128, 4], F32)
            wz = wload(Wz, [128, 4 * NZ], F32R)
            n2mu = wload(NEG2MUT, [NZ, KE], F32R)
            o64 = wload(ONES64, [NZ, KE], F32R)
            o16 = wload(ONES16, [KE, 1], F32R)
            o132 = wload(ONES1_32, [1, 32], F32R)
            onesn = wload(ONESN, [1, NB], F32R)
            bz = wload(BZ, [NZ, 1], F32)
            bd1 = wload(BD1, [1, KE], F32R)

            late = {}

            def load_late_weights():
                late["w1p"] = wload(W1p, [128, NPAIR * H_EXP], F32R)
                late["w2c"] = wload(W2c, [128, KE * 2 * 32], F32R)
                late["e2"] = wload(E2, [KE, NPAIR * 128], F32R)
                late["b2p"] = wload(B2PAD, [KE, 32], F32R)
                if has_b1:
                    late["b1c"] = wload(B1C, [128, KE * 2], F32)
                    late["e2s"] = wload(E2S, [KE, KE * 128], F32R)

            def front(ib):
                n0 = ib * NB
                xt = xp.tile([128, 8 * NB], F32R, tag="xt")
                for dc in range(8):
                    nc.sync.dma_start(
                        xt[:, dc * NB:dc * NB + W], XT[dc, :, n0:n0 + W]
                    )

                hts = []
                for hc in range(4):
                    ph = pbig.tile([128, NB], F32, tag="pbig")
                    for dc in range(8):
                        nc.tensor.matmul(
                            ph[:, :W],
                            wenc[:, dc * H_ENC + hc * 128: dc * H_ENC + (hc + 1) * 128],
                            xt[:, dc * NB:dc * NB + W],
                            start=(dc == 0), stop=(dc == 7),
                        )
                    ht = hp.tile([128, NB], F32R, tag="ht")
                    nc.scalar.activation(ht[:, :W], ph[:, :W], AF.Relu,
                                         bias=benc[:, hc:hc + 1])
                    hts.append(ht)

                pz = pmisc.tile([NZ, NB], F32, tag="pmisc")
                for hc in range(4):
                    nc.tensor.matmul(
                        pz[:, :W], wz[:, hc * NZ:(hc + 1) * NZ], hts[hc][:, :W],
                        start=(hc == 0), stop=(hc == 3),
                    )
                zt2 = zp.tile([128, NB], F32R, tag="zt2")
                nc.scalar.activation(zt2[0:NZ, :W], pz[:, :W], AF.Identity, bias=bz[:])
                nc.scalar.activation(zt2[NZ:128, :W], pz[:, :W], AF.Identity, bias=bz[:])
                zsq = zp.tile([NZ, NB], F32R, tag="zsq")
                nc.vector.tensor_mul(zsq[:, :W], zt2[0:NZ, :W], zt2[0:NZ, :W])

                pd = pmisc.tile([KE, NB], F32, tag="pmisc")
                nc.tensor.matmul(pd[:, :W], bd1[:], onesn[:, :W], start=True, stop=False)
                nc.tensor.matmul(pd[:, :W], n2mu[:], zt2[0:NZ, :W], start=False, stop=False)
                nc.tensor.matmul(pd[:, :W], o64[:], zsq[:, :W], start=False, stop=True)
                qr = qp.tile([KE, NB], F32R, tag="qr")
                nc.vector.reciprocal(qr[:, :W], pd[:, :W])
                pqs = pmisc.tile([1, NB], F32, tag="pmisc")
                nc.tensor.matmul(pqs[:, :W], o16[:], qr[:, :W], start=True, stop=True)
                rqs = qp.tile([1, NB], F32R, tag="rqs")
                nc.vector.reciprocal(rqs[:, :W], pqs[:, :W])
                prb = pmisc.tile([32, NB], F32, tag="pmisc")
                nc.tensor.matmul(prb[:, :W], o132[:], rqs[:, :W], start=True, stop=True)
                prb_sb = qp.tile([32, NB], F32R, tag="prb_sb")
                nc.scalar.activation(prb_sb[:, :W], prb[:, :W], AF.Copy)
                return dict(zt2=zt2, qr=qr, prb_sb=prb_sb, n0=n0)

            def back(st):
                zt2, qr, prb_sb, n0 = st["zt2"], st["qr"], st["prb_sb"], st["n0"]
                pp = ppred.tile([32, NB], F32, tag="ppred")
                nc.tensor.matmul(pp[:, :W], late["b2p"][:], qr[:, :W], start=True, stop=False)

                ncomb = NPAIR * 4
                ci = 0
                for j in range(NPAIR):
                    if not has_b1:
                        pqbt = pqb.tile([128, NB], F32, tag="pqb")
                        nc.tensor.matmul(pqbt[:, :W], late["e2"][:, j * 128:(j + 1) * 128],
                                         qr[:, :W], start=True, stop=True)
                        zq = zqp.tile([128, NB], F32R, tag="zq")
                        nc.vector.tensor_mul(zq[:, :W], zt2[:, :W], pqbt[:, :W])
                    else:
                        zq = zt2
                    for hc in range(2):
                        for half in range(2):
                            k = 2 * j + half
                            idx = k * 2 + hc
                            pe_ = pbig.tile([128, NB], F32, tag="pbig")
                            nc.tensor.matmul(
                                pe_[:, :W],
                                late["w1p"][64 * half:64 * (half + 1),
                                    j * H_EXP + hc * 128: j * H_EXP + (hc + 1) * 128],
                                zq[64 * half:64 * (half + 1), :W],
                                start=True, stop=True,
                                tile_position=(64 * half, 0),
                            )
                            eh = ehp.tile([128, NB], F32R, tag="eh")
                            if not has_b1:
                                if idx % 8 < 5:
                                    nc.scalar.activation(eh[:, :W], pe_[:, :W], AF.Relu,
                                                         bias=0.0)
                                else:
                                    nc.vector.tensor_scalar_max(eh[:, :W], pe_[:, :W], 0.0)
                            else:
                                nc.scalar.activation(eh[:], pe_[:], AF.Relu,
                                                     bias=late["b1c"][:, idx:idx + 1])
                                pqk = pqb.tile([128, NB], F32, tag="pqb")
                                nc.tensor.matmul(pqk[:],
                                                 late["e2s"][:, k * 128:(k + 1) * 128],
                                                 qr[:], start=True, stop=True)
                                ehq = ehp.tile([128, NB], F32R, tag="ehq")
                                nc.vector.tensor_mul(ehq[:], eh[:], pqk[:])
                                eh = ehq
                            ci += 1
                            nc.tensor.matmul(
                                pp[:, :W],
                                late["w2c"][:, idx * 32:(idx + 1) * 32],
                                eh[:, :W],
                                start=False, stop=(ci == ncomb),
                                skip_group_check=True,
                            )

                ti = trp.tile([32, NB], F32, tag="ti")
                nc.scalar.activation(ti[:, :W], pp[:, :W], AF.Copy)
                nc.vector.tensor_mul(ti[:, :W], ti[:, :W], prb_sb[:, :W])
                tr = trp.tile([32, NB], F32, tag="tr")
                nc.vector.transpose(tr[:, :W], ti[:, :W])
                nc.sync.dma_start(
                    OUT[n0:n0 + W, :].rearrange("(b p) c -> p b c", p=32),
                    tr[:].rearrange("p (b v) -> p b v", v=32)[:, 0:W // 32, 0:C],
                )

            A = cfg["ahead"]
            for _rep in range(cfg["repeat"]):
                sts = [front(0)]
                if _rep == 0 and not late:
                    load_late_weights()
                for ib in range(1, min(A, NBLK)):
                    sts.append(front(ib))
                for ib in range(NBLK):
                    if ib + A < NBLK:
                        sts.append(front(ib + A))
                    back(sts[ib])
                sts.clear()

    nc.compile()
    return nc


def _prep_legacy(inputs):
    f = lambda a: np.ascontiguousarray(np.asarray(a, dtype=np.float32))
    X, enc_W, enc_b = f(inputs["X"]), f(inputs["enc_W"]), f(inputs["enc_b"])
    z_W, z_b, mu = f(inputs["z_W"]), f(inputs["z_b"]), f(inputs["mu"])
    W1, b1, W2, b2 = f(inputs["W1"]), f(inputs["b1"]), f(inputs["W2"]), f(inputs["b2"])

    has_b1 = bool(np.any(b1))

    XT = np.ascontiguousarray(X.T)
    com = {
        "Wenc": np.ascontiguousarray(
            enc_W.reshape(8, 128, H_ENC).transpose(1, 0, 2).reshape(128, 8 * H_ENC)),
        "Wz": np.ascontiguousarray(
            z_W.reshape(4, 128, NZ).transpose(1, 0, 2).reshape(128, 4 * NZ)),
        "NEG2MUT": np.ascontiguousarray(-2.0 * mu.T),
        "ONES64": np.ones((NZ, KE), np.float32),
        "ONES16": np.ones((KE, 1), np.float32),
        "ONES1_32": np.ones((1, 32), np.float32),
        "ONESN": np.ones((1, NB), np.float32),
        "BENC": np.ascontiguousarray(enc_b.reshape(4, 128).T),
        "BZ": z_b.reshape(NZ, 1).copy(),
        "BD1": (1.0 + (mu.astype(np.float64) ** 2).sum(axis=1)
                ).astype(np.float32).reshape(1, KE),
    }
    w1p = np.zeros((128, NPAIR * H_EXP), np.float32)
    e2 = np.zeros((KE, NPAIR * 128), np.float32)
    for j in range(NPAIR):
        w1p[0:64, j * H_EXP:(j + 1) * H_EXP] = W1[2 * j]
        w1p[64:128, j * H_EXP:(j + 1) * H_EXP] = W1[2 * j + 1]
        e2[2 * j, j * 128: j * 128 + 64] = 1.0
        e2[2 * j + 1, j * 128 + 64: j * 128 + 128] = 1.0
    com["W1p"], com["E2"] = w1p, e2

    w2c = np.zeros((128, KE * 2 * 32), np.float32)
    for k in range(KE):
        for hc in range(2):
            w2c[:, (k * 2 + hc) * 32:(k * 2 + hc) * 32 + C] = \
                W2[k][hc * 128:(hc + 1) * 128, :]
    com["W2c"] = w2c

    b2pad = np.zeros((KE, 32), np.float32)
    b2pad[:, 0:C] = b2
    com["B2PAD"] = b2pad

    if has_b1:
        b1c = np.zeros((128, KE * 2), np.float32)
        e2s = np.zeros((KE, KE * 128), np.float32)
        for k in range(KE):
            for hc in range(2):
                b1c[:, k * 2 + hc] = b1[k, hc * 128:(hc + 1) * 128]
            e2s[k, k * 128:(k + 1) * 128] = 1.0
        com["B1C"], com["E2S"] = b1c, e2s

    in_maps = []
    for c in range(NCORES):
        m = dict(com)
        shard = np.ascontiguousarray(XT[:, c * NS:(c + 1) * NS])
        m["XT"] = shard.reshape(8, 128, NS)
        in_maps.append(m)
    return in_maps, has_b1
